# revision 19
# baseline (speedup 1.0000x reference)
"""Multi-head attention Bass/Tile kernel for Trainium2, 8-way sharded.

Problem: nn_MultiHeadAttention (B=4, S=2048, d_model=768, H=12, d_k=64).

Sharding (data parallel x tensor parallel, per the head-split hint):
core c handles batch b=c//2 and head group hg=c%2 (6 of 12 heads). Each core
projects Q/K/V only for its heads (weight columns sliced host-side), runs
attention for its heads over the full sequence, and computes a partial
W_o projection (contraction over its heads' features). The two partials per
batch are summed during the host-side gather — the "all-reduce after W_o".

On-chip dataflow (per core), matmuls bf16 with fp32 PSUM accumulation:
  - q/k/v arrive bf16 AND pre-transposed to feature-major [d, t] from the
    host: on-chip loads are plain contiguous DMAs, no xbar transposes.
  - Q^T, K^T projections produce feature-major outputs; V is token-major
    with an extra all-ones column per head so the P@V matmul also
    accumulates softmax row-sums.
  - Scores are computed transposed (S^T[k, q]); the K=64 contraction only
    uses half the PE array, so the two heads of a pair (partitions 0-63 /
    64-127) are issued as ADJACENT matmuls on row-tiles T0/T8 of the
    64x128 tiling mode — they execute concurrently (~2x score throughput).
  - Softmax-exp runs on ScalarE straight out of PSUM (1/sqrt(dk) fused
    into the activation); no max-subtraction (scores are N(0,1)-scale).
    ScalarE is the pipeline governor (~220us of exp), so score groups are
    emitted early and continuously.
  - Row-sum reciprocals are broadcast across feature partitions via a tiny
    fp32r selection-matrix matmul; normalization fused into PSUM->SBUF
    eviction of the context.
  - bv/bo fold host-side into bo' = bv @ Wo + bo, applied via a rank-1
    ones-row matmul on the hg=0 cores only.
"""

import numpy as np

import concourse.bass as bass
import concourse.tile as tile
from concourse import bacc, mybir

F32 = mybir.dt.float32
F32R = mybir.dt.float32r
BF16 = mybir.dt.bfloat16
FP8 = mybir.dt.float8e4

# exp(x) is emitted as exp(x - ln4): keeps P below fp8e4's max finite value
# (240) for N(0,1)-scale scores; softmax normalization cancels the constant.
EXP_BIAS = -1.3862943611198906


def build_mha(nc, SQ, SK, D, DO, DK, compile_=True):
    """Emit the per-core MHA program. D = model width (contraction for
    QKV projections), DO = this core's head-feature width (H_loc * DK)."""
    DT = D // 128           # input feature tiles (contraction)
    DTO = DO // 128         # local head-feature tiles == head PAIRS
    HPD = 128 // DK         # heads per feature tile (2)
    H = DTO * HPD           # local heads
    NP = DTO                # head pairs; pair j = heads (2j, 2j+1) = dt j
    assert H * DK == DO and DO <= 512 and HPD == 2
    KT = SK // 128          # key token tiles
    TCH = min(1024, SQ, SK)  # token chunk for input loads/projections
    KTC = TCH // 128        # k-tiles per chunk
    QCH = min(512, SQ)      # query chunk for attention
    NQC = SQ // QCH
    NKC = SK // TCH         # k/v chunks
    NQCH = SQ // TCH        # q chunks
    NFC = (D + 511) // 512  # out-proj feature chunks
    FCH = D // NFC
    VW = 72                 # V cols per head: DK data + ones col + pad so the
                            # DoubleRow ko stride H*VW is 16B-aligned
    G = 2                   # k-tiles per exp group

    q_in = nc.dram_tensor("qT_in", [D, SQ], BF16, kind="ExternalInput").ap()
    k_in = nc.dram_tensor("kT_in", [D, SK], BF16, kind="ExternalInput").ap()
    v_in = nc.dram_tensor("vT_in", [D, SK], BF16, kind="ExternalInput").ap()
    Wq_ = nc.dram_tensor("Wq", [D, DO], BF16, kind="ExternalInput").ap()
    Wk_ = nc.dram_tensor("Wk", [D, DO], BF16, kind="ExternalInput").ap()
    Wv_ = nc.dram_tensor("Wv", [D, DO], BF16, kind="ExternalInput").ap()
    Wo_ = nc.dram_tensor("Wo", [DO, D], BF16, kind="ExternalInput").ap()
    bq_ = nc.dram_tensor("bq", [DO], F32, kind="ExternalInput").ap()
    bk_ = nc.dram_tensor("bk", [DO], F32, kind="ExternalInput").ap()
    bo2_ = nc.dram_tensor("bo2", [D], BF16, kind="ExternalInput").ap()
    sel_ = nc.dram_tensor("sel_in", [HPD, 128], F32R, kind="ExternalInput").ap()
    out_ = nc.dram_tensor("out", [SQ, D], F32, kind="ExternalOutput").ap()

    with tile.TileContext(nc) as tc, \
            tc.tile_pool(name="persist", bufs=1) as persist, \
            tc.tile_pool(name="p_inT", bufs=2) as p_inT, \
            tc.tile_pool(name="b_p", bufs=4) as b_p, \
            tc.tile_pool(name="b_sm", bufs=2) as b_sm, \
            tc.tile_pool(name="b_out", bufs=2) as b_out, \
            tc.tile_pool(name="b_s", bufs=2, space="PSUM") as b_s, \
            tc.tile_pool(name="b_pv", bufs=2, space="PSUM") as b_pv, \
            tc.tile_pool(name="b_misc", bufs=2, space="PSUM") as b_misc:
        scale = 1.0 / float(np.sqrt(np.float32(DK)))

        # --- constants + weights via SWDGE (sync HWDGE queue reserved for
        # the bulk input-chunk loads) ---
        ones_row = persist.tile([1, 128], BF16)
        nc.vector.memset(ones_row[:], 1.0)
        sel = persist.tile([HPD, 128], F32R)
        nc.gpsimd.dma_start(out=sel[:], in_=sel_[:])
        bq_sb = persist.tile([128, DTO], F32)
        nc.gpsimd.dma_start(out=bq_sb[:], in_=bq_.rearrange("(dt p) -> p dt", p=128))
        bk_sb = persist.tile([128, DTO], F32)
        nc.gpsimd.dma_start(out=bk_sb[:], in_=bk_.rearrange("(dt p) -> p dt", p=128))
        bo2_sb = persist.tile([1, D], BF16)
        nc.gpsimd.dma_start(out=bo2_sb[:], in_=bo2_[None, :])

        w_sb = {}
        for name, ap in (("Wk", Wk_), ("Wq", Wq_), ("Wv", Wv_)):
            t = persist.tile([128, DT, DO], BF16, name=f"{name}_sb")
            nc.gpsimd.dma_start(
                out=t[:], in_=ap.rearrange("(dt p) f -> p dt f", p=128)
            )
            w_sb[name] = t
        wo_sb = persist.tile([128, DTO, D], BF16, name="Wo_sb")
        nc.gpsimd.dma_start(
            out=wo_sb[:], in_=Wo_.rearrange("(dt p) f -> p dt f", p=128)
        )

        # --- persistent activations ---
        Q_sb = persist.tile([128, DTO, SQ], BF16)    # Q^T feature-major
        K_sb = persist.tile([128, DTO, SK], BF16)    # K^T feature-major
        V_sb = persist.tile([128, KT, H, VW], BF16)  # V token-major + ones
        nc.vector.memset(V_sb[:, :, :, DK : DK + 1], 1.0)
        xn_sb = persist.tile([128, DTO, SQ], BF16)   # normalized context^T

        def load_transposed_chunk(src, c, parts=1):
            """Load a feature-major [128, DT, TCH] chunk from the
            pre-transposed bf16 DRAM tensor [D, S] (plain contiguous DMA).
            parts>1 splits the load token-wise for earlier availability."""
            inT = p_inT.tile([128, DT, TCH], BF16, tag="inT")
            srcr = src.rearrange("(dt p) t -> p dt t", p=128)
            step = TCH // parts
            for i in range(parts):
                lo = c * TCH + i * step
                nc.sync.dma_start(
                    out=inT[:, :, i * step : (i + 1) * step],
                    in_=srcr[:, :, lo : lo + step],
                )
            return inT

        def emit_qk_proj(inT, c, W, bias_sb, dst_sb, only=None):
            """Feature-major projection chunk: dst[f, t] = W^T . inT + b.
            only: optional (dtf, sub) filter for micro-slicing."""
            SUB = min(512, TCH)
            for dtf in range(DTO):
                for sub in range(TCH // SUB):
                    if only is not None and (dtf, sub) != only:
                        continue
                    pk = b_misc.tile([128, SUB], F32, tag="misc")
                    for dtd in range(DT):
                        nc.tensor.matmul(
                            pk[:],
                            W[:, dtd, dtf * 128 : (dtf + 1) * 128],
                            inT[:, dtd, sub * SUB : (sub + 1) * SUB],
                            start=(dtd == 0),
                            stop=(dtd == DT - 1),
                        )
                    nc.vector.tensor_scalar_add(
                        dst_sb[:, dtf, c * TCH + sub * SUB : c * TCH + (sub + 1) * SUB],
                        pk[:],
                        bias_sb[:, dtf : dtf + 1],
                    )

        def emit_v_proj(inT, c, only=None):
            """Token-major V projection with per-head column interleave."""
            for tt in range(KTC):
                if only is not None and tt not in only:
                    continue
                kt = c * KTC + tt
                pv = b_misc.tile([128, DO], F32, tag="misc")
                for dtd in range(DT):
                    nc.tensor.matmul(
                        pv[:],
                        inT[:, dtd, tt * 128 : (tt + 1) * 128],
                        w_sb["Wv"][:, dtd, :],
                        start=(dtd == 0),
                        stop=(dtd == DT - 1),
                    )
                nc.vector.tensor_copy(
                    V_sb[:, kt, :, 0:DK],
                    pv[:].rearrange("p (h d) -> p h d", d=DK),
                )

        def emit_pair_scores(j, qc, P_A, P_B, g_lo, g_hi):
            """Scores + exp for groups [g_lo, g_hi) of pair j, chunk qc.
            The two heads' score matmuls are emitted ADJACENT so they run
            concurrently on row-tiles T0 (partitions 0-63) and T8 (64-127)
            of the 64x128 tiling mode."""
            q0 = qc * QCH
            for g in range(g_lo, g_hi):
                ps_a = b_s.tile([128, G, QCH], F32, tag="s")
                ps_b = b_s.tile([128, G, QCH], F32, tag="s")
                for i in range(G):
                    kt = g * G + i
                    k0 = kt * 128
                    nc.tensor.matmul(
                        ps_a[:, i],
                        K_sb[0:DK, j, k0 : k0 + 128],
                        Q_sb[0:DK, j, q0 : q0 + QCH],
                        start=True,
                        stop=True,
                    )
                    nc.tensor.matmul(
                        ps_b[:, i],
                        K_sb[DK:128, j, k0 : k0 + 128],
                        Q_sb[DK:128, j, q0 : q0 + QCH],
                        start=True,
                        stop=True,
                    )
                nc.scalar.activation(
                    P_A[:, g * G : (g + 1) * G, :], ps_a[:],
                    mybir.ActivationFunctionType.Exp, scale=scale,
                )
                nc.scalar.activation(
                    P_B[:, g * G : (g + 1) * G, :], ps_b[:],
                    mybir.ActivationFunctionType.Exp, scale=scale,
                )

        def emit_pair_pv(j, P_A, P_B, ppv_a, ppv_b, kt_lo, kt_hi):
            for kt in range(kt_lo, kt_hi):
                nc.tensor.matmul(
                    ppv_a[:],
                    V_sb[:, kt, 2 * j, 0 : DK + 1],
                    P_A[:, kt, :],
                    start=(kt == 0),
                    stop=(kt == KT - 1),
                )
                nc.tensor.matmul(
                    ppv_b[:],
                    V_sb[:, kt, 2 * j + 1, 0 : DK + 1],
                    P_B[:, kt, :],
                    start=(kt == 0),
                    stop=(kt == KT - 1),
                )

        def emit_steady_pair(j, qc, nxt):
            """One steady-state step: PV+tail of pair (j, qc) with the NEXT
            pair's score groups interleaved at group granularity, so the PE
            always has 128-mode PV work during Act's exp of each group and
            vice versa. nxt = (nj, nqc, NP_A, NP_B) or None."""
            P_A, P_B = P_tiles.pop(j)
            ppv_a = b_pv.tile([DK + 1, QCH], F32, tag="pv")
            ppv_b = b_pv.tile([DK + 1, QCH], F32, tag="pv")
            for g in range(NG):
                if nxt is not None:
                    emit_pair_scores(nxt[0], nxt[1], nxt[2], nxt[3], g, g + 1)
                emit_pair_pv(j, P_A, P_B, ppv_a, ppv_b, G * g, G * g + G)
                if j == 0 and pending_out[0] is not None and g % 2 == 1:
                    emit_outproj(pending_out[0], tts=[g // 2])
                    if g == NG - 1:
                        pending_out[0] = None
            emit_pair_tail(j, qc, ppv_a, ppv_b)

        def emit_pair_tail(j, qc, ppv_a, ppv_b):
            """Evict pair j's contexts + rowsums, then normalize feature
            tile j of chunk qc (pair j IS feature tile j)."""
            q0 = qc * QCH
            xraw = b_sm.tile([128, QCH], F32, tag="xraw")
            rT = b_sm.tile([HPD, QCH], F32, tag="rT")
            for hh, ppv in ((0, ppv_a), (1, ppv_b)):
                rh = b_sm.tile([1, QCH], F32, tag="rh")
                nc.vector.tensor_copy(rh[:], ppv[DK : DK + 1, :])
                # DMA scatter: engines can't write partition base hh, DMA can
                nc.gpsimd.dma_start(out=rT[hh : hh + 1, :], in_=rh[:])
                nc.vector.tensor_copy(xraw[hh * DK : (hh + 1) * DK, :], ppv[0:DK, :])
            rinv = b_sm.tile([HPD, QCH], F32R, tag="rinv")
            rtmp = b_sm.tile([HPD, QCH], F32, tag="rtmp")
            with nc.allow_low_precision(reason="f32r softmax-normalizer bcast"):
                nc.vector.reciprocal_approx_fast(rtmp[:], rT[:])
                nc.vector.tensor_copy(rinv[:], rtmp[:])
            pb = b_misc.tile([128, QCH], F32, tag="misc")
            nc.tensor.matmul(pb[:], sel[:], rinv[:], start=True, stop=True)
            nc.vector.tensor_mul(
                xn_sb[:, j, q0 : q0 + QCH], xraw[:], pb[:]
            )

        def emit_outproj(qc, tts=None):
            q0 = qc * QCH
            for tt in tts if tts is not None else range(QCH // 128):
                t0 = q0 + tt * 128
                ob = b_out.tile([128, D], F32, tag="ob")
                for fch in range(NFC):
                    po = b_misc.tile([128, FCH], F32, tag="misc")
                    for dtd in range(DTO):
                        nc.tensor.matmul(
                            po[:],
                            xn_sb[:, dtd, t0 : t0 + 128],
                            wo_sb[:, dtd, fch * FCH : (fch + 1) * FCH],
                            start=(dtd == 0),
                            stop=False,
                        )
                    nc.tensor.matmul(
                        po[:],
                        ones_row[:],
                        bo2_sb[:, fch * FCH : (fch + 1) * FCH],
                        start=False,
                        stop=True,
                    )
                    nc.vector.tensor_copy(ob[:, fch * FCH : (fch + 1) * FCH], po[:])
                nc.sync.dma_start(out=out_[t0 : t0 + 128, :], in_=ob[:])

        # ---------------- emission schedule ----------------
        # ScalarE (exp) is the governor: ~1.15us per score group vs ~0.5us
        # for the PE to produce one. Emit K/Q projections dtf-by-dtf with
        # each pair's first score groups right behind, so Act ramps within
        # a few us; thereafter keep score groups flowing ahead of V-proj/
        # PV/out-proj filler work on the PE.
        NGC = KTC // G          # score groups per k-chunk (4)
        NG = KT // G            # score groups total (8)

        kT0 = load_transposed_chunk(k_in, 0, parts=2)
        qT0 = load_transposed_chunk(q_in, 0, parts=2)

        # P tiles for qc0 primers (3 pairs in flight needs bufs=3 per slot
        # set; A/B of one pair + A of the next fit in b_p bufs=3... use
        # dedicated tags per pair parity to keep slots stable)
        P_tiles = {}
        _p_ctr = [0]

        def p_tile():
            _p_ctr[0] += 1
            return b_p.tile([128, KT, QCH], BF16, tag="P", name=f"P{_p_ctr[0]}")

        # --- qc0 primers, chunk-0 k-tiles: proj slices then scores.
        # Only pair 0 is primed here (b_p slot budget); pair 1+ prime
        # inside the steady loop, interleaved with the previous pair's PV.
        for j in range(NP):
            emit_qk_proj(kT0, 0, w_sb["Wk"], bk_sb, K_sb, only=(j, 0))
            emit_qk_proj(kT0, 0, w_sb["Wk"], bk_sb, K_sb, only=(j, 1))
            emit_qk_proj(qT0, 0, w_sb["Wq"], bq_sb, Q_sb, only=(j, 0))
            if j < 1:
                P_A, P_B = p_tile(), p_tile()
                P_tiles[j] = (P_A, P_B)
                emit_pair_scores(j, 0, P_A, P_B, 0, NGC)
        # Q proj chunk0 remaining subs (tokens 512-1023 = qc1)
        for j in range(NP):
            emit_qk_proj(qT0, 0, w_sb["Wq"], bq_sb, Q_sb, only=(j, 1))

        vT0 = load_transposed_chunk(v_in, 0)
        emit_v_proj(vT0, 0)
        kT1 = load_transposed_chunk(k_in, 1)
        for j in range(NP):
            emit_qk_proj(kT1, 1, w_sb["Wk"], bk_sb, K_sb, only=(j, 0))
            emit_qk_proj(kT1, 1, w_sb["Wk"], bk_sb, K_sb, only=(j, 1))
            if j in P_tiles:
                P_A, P_B = P_tiles[j]
                emit_pair_scores(j, 0, P_A, P_B, NGC, NG)
        vT1 = load_transposed_chunk(v_in, 1)
        emit_v_proj(vT1, 1)
        qT1 = load_transposed_chunk(q_in, 1)
        emit_qk_proj(qT1, 1, w_sb["Wq"], bq_sb, Q_sb)

        # --- steady state over (qc, pair) ---
        # Per step: PV of the current pair interleaved group-wise with the
        # next pair's scores (which feed Act); out-proj of qc-1 as filler.
        pending_out = [None]
        for qc in range(NQC):
            for j in range(NP):
                nj, nqc = (j + 1, qc) if j + 1 < NP else (0, qc + 1)
                nxt = None
                if nqc < NQC and nj not in P_tiles:
                    NP_A, NP_B = p_tile(), p_tile()
                    P_tiles[nj] = (NP_A, NP_B)
                    nxt = (nj, nqc, NP_A, NP_B)
                emit_steady_pair(j, qc, nxt)
            pending_out[0] = qc
        emit_outproj(pending_out[0])

    if compile_:
        nc.compile()
    return nc


# ------------------------- host-side entry point -------------------------

D_MODEL = 768
N_HEADS = 12
D_K = 64
B_FULL, S_FULL = 4, 2048
N_CORES = 8
HEAD_SPLIT = 2                      # head groups (tensor parallel)
DO_CORE = D_MODEL // HEAD_SPLIT     # per-core head-feature width

_cached_nc = None


def _make_sel(HPD, DK):
    """sel[j, p] = 1 iff partition p belongs to pair-member j (p//DK == j)."""
    sel = np.zeros((HPD, HPD * DK), dtype=np.float32)
    for j in range(HPD):
        sel[j, j * DK : (j + 1) * DK] = 1.0
    return sel


def _get_nc():
    global _cached_nc
    if _cached_nc is None:
        nc = bacc.Bacc("TRN2", target_bir_lowering=False, debug=False)
        build_mha(nc, SQ=S_FULL, SK=S_FULL, D=D_MODEL, DO=DO_CORE, DK=D_K)
        _cached_nc = nc
    return _cached_nc


def kernel(q, k, v, Wq, bq, Wk, bk, Wv, bv, Wo, bo, _trace=False, _tmpdir=None):
    from concourse.bass_utils import run_bass_kernel_spmd
    import ml_dtypes

    bf16 = ml_dtypes.bfloat16
    q = np.ascontiguousarray(np.asarray(q, dtype=np.float32))
    k = np.ascontiguousarray(np.asarray(k, dtype=np.float32))
    v = np.ascontiguousarray(np.asarray(v, dtype=np.float32))
    Wq, Wk, Wv, Wo = (
        np.ascontiguousarray(np.asarray(w, dtype=np.float32)) for w in (Wq, Wk, Wv, Wo)
    )
    bq, bk, bv, bo = (np.asarray(x, dtype=np.float32) for x in (bq, bk, bv, bo))
    B, S, D = q.shape
    assert (B, S, D) == (B_FULL, S_FULL, D_MODEL), (B, S, D)

    # fold bv, bo into a single output-side bias: softmax rows sum to 1 so
    # context_with_bv = context + bv  =>  out = ctx @ Wo + (bv @ Wo + bo).
    # Applied only on the hg=0 partial of each batch pair.
    bo2 = (bv.astype(np.float32) @ Wo + bo).astype(bf16)
    bo2_zero = np.zeros_like(bo2)
    sel_np = _make_sel(128 // D_K, D_K)

    qT16 = [np.ascontiguousarray(q[b].T.astype(bf16)) for b in range(B)]
    kT16 = [np.ascontiguousarray(k[b].T.astype(bf16)) for b in range(B)]
    vT16 = [np.ascontiguousarray(v[b].T.astype(bf16)) for b in range(B)]
    W16 = {
        "Wq": Wq.astype(bf16), "Wk": Wk.astype(bf16),
        "Wv": Wv.astype(bf16), "Wo": Wo.astype(bf16),
    }

    in_maps = []
    for c in range(N_CORES):
        b, hg = divmod(c, HEAD_SPLIT)
        f0, f1 = hg * DO_CORE, (hg + 1) * DO_CORE
        in_maps.append(
            {
                "qT_in": qT16[b],
                "kT_in": kT16[b],
                "vT_in": vT16[b],
                "Wq": np.ascontiguousarray(W16["Wq"][:, f0:f1]),
                "Wk": np.ascontiguousarray(W16["Wk"][:, f0:f1]),
                "Wv": np.ascontiguousarray(W16["Wv"][:, f0:f1]),
                "Wo": np.ascontiguousarray(W16["Wo"][f0:f1, :]),
                "bq": np.ascontiguousarray(bq[f0:f1]),
                "bk": np.ascontiguousarray(bk[f0:f1]),
                "bo2": bo2 if hg == 0 else bo2_zero,
                "sel_in": sel_np,
            }
        )

    nc = _get_nc()
    res = run_bass_kernel_spmd(
        nc, in_maps, core_ids=list(range(N_CORES)), trace=_trace, tmpdir=_tmpdir
    )

    # gather/unshard: sum the two head-group partials per batch (the
    # "all-reduce after W_o" of the tensor-parallel head split)
    out = np.empty((B, S, D), dtype=np.float32)
    for b in range(B):
        out[b] = res.results[b * HEAD_SPLIT]["out"]
        for hg in range(1, HEAD_SPLIT):
            out[b] += res.results[b * HEAD_SPLIT + hg]["out"]
    kernel._last_exec_time_ns = res.exec_time_ns
    return out
